# revision 24
# baseline (speedup 1.0000x reference)
"""Trainium2 Bass kernel for nn_LocalExperts (MoE expert-parallel FFN), v2.

Reference computation (per full input):
    x  [T=16384, D=1024] -> reshape [E=8, C=2048, D]
    h  = gelu(x @ w1[e] + b1[e])     w1 [E, D, F=4096]
    y  = h @ w2[e] + b2[e]           w2 [E, F, D]
    out[T, D]

Sharding: expert parallelism across 8 NeuronCores. Expert e's tokens are
exactly rows [e*C:(e+1)*C] of the input, so core e gets that token slice
plus w1[e], b1[e], w2[e], b2[e]. No collectives; outputs are concatenated
on the host.

Design notes (evolved from a 592us f32r two-pass baseline to ~460us):
  - Everything 16-bit on the wire: x/w1/w2 are cast to bf16 host-side
    (biases, PSUM accumulation and y stay f32; end-to-end rel err ~3e-3
    vs the 2e-2 gate). bf16 matmuls run the PE at full rate AND get the
    compiler's fast-weight-load, so LDWEIGHTS (~97ns) hides under the
    512-row stream -- f32r's ~187ns loads did not quite hide.
  - Host prep also pre-transposes x and packs x/w1/w2 chunk-major so
    every DMA lands 2-8KB contiguous per partition; small-packet DMAs
    (256B) measured only ~63 GB/s and starved the first GEMM chain.
  - Single token pass, F chunked 8 x 512: weights are DMA'd exactly once
    (20MB total HBM reads vs 76MB for the two-pass f32r version).
  - GEMM2 accumulates chunk PAIRS in one PSUM chain (8 f-tiles) before
    the DVE drains into Yacc: with per-chunk drains the DVE rate-matched
    the PE and beat against it (432ns PE stall every ~49 matmuls).
  - Startup: the ACT queue carries only the two tiny loads the first
    GELU drain needs (fused b1t|b2b consts + w1 chunk 0's first f-tile);
    bulk streams on sync. A few warm matmuls ramp the HAM clock
    (1.2->2.4GHz) while the first Xt quarter lands.
"""

import os
from contextlib import ExitStack

import ml_dtypes
import numpy as np

import concourse.bass as bass
import concourse.tile as tile
from concourse import bacc
from concourse import mybir
from concourse.bass import ds, ts
from concourse.bass_utils import run_bass_kernel_spmd

AFT = mybir.ActivationFunctionType

E = 8
D = 1024
F = 4096
T = 16384
C = T // E          # tokens per core
P = 128

FC = 512            # F chunk per iteration
N_FC = F // FC      # 8 chunks
FC_T = FC // P      # 4 f-tiles per chunk
D_T = D // P        # 8 d-tiles
C_B = C // P        # 16 token blocks
NQ = 4              # token quarters (GEMM1 moving dim 512)
QW = C // NQ
N_WARM = int(os.environ.get("KERNEL_WARM", "21"))  # HAM clock warm-up matmuls

# test-only: CoreSim lacks Gelu; "tanh" swaps the activation for sim gating
ACT_FN = os.environ.get("KERNEL_ACT", "gelu")


def _emit(ctx: ExitStack, tc: tile.TileContext, xtp, w1p, w2p, cb, y):
    nc = tc.nc
    f32 = mybir.dt.float32
    bf16 = mybir.dt.bfloat16

    consts = ctx.enter_context(tc.tile_pool(name="consts", bufs=1))
    xt_pool = ctx.enter_context(tc.tile_pool(name="xt", bufs=1))
    yacc_pool = ctx.enter_context(tc.tile_pool(name="yacc", bufs=1))
    w1_pool = ctx.enter_context(tc.tile_pool(name="w1c", bufs=2))
    w2_pool = ctx.enter_context(tc.tile_pool(name="w2c", bufs=3))
    ht_pool = ctx.enter_context(tc.tile_pool(name="ht", bufs=3))
    mm1_psum = ctx.enter_context(tc.tile_pool(name="mm1", bufs=3, space="PSUM"))
    mm2_psum = ctx.enter_context(tc.tile_pool(name="mm2", bufs=4, space="PSUM"))

    # Warm the PE HAM clock while the first Xt quarter + w1 chunk DMA in.
    if N_WARM:
        dummy = consts.tile([P, QW], bf16)
        nc.gpsimd.memset(dummy[:], 0.0)
        for _ in range(N_WARM):
            warm_ps = mm1_psum.tile([P, QW], f32, tag="mm1", name="warm_ps")
            nc.tensor.matmul(warm_ps[:], lhsT=dummy[:, :P], rhs=dummy[:],
                             start=True, stop=True)

    xt = xt_pool.tile([P, NQ, D_T, QW], bf16, tag="xt")
    yacc = yacc_pool.tile([P, C_B, D], f32, tag="yacc")
    # fused consts: [:, :F//P] = b1 packed per f-tile, [:, F//P:] = b2
    # broadcast -- one 4.2KB-per-partition DMA instead of two small ones.
    cbt = consts.tile([P, F // P + D], f32)
    b1t = cbt[:, : F // P]
    b2b = cbt[:, F // P :]

    w1cs = [None] * N_FC
    w2cs = [None] * N_FC

    def load_chunk(k, q):
        w1cs[k] = w1_pool.tile([P, FC_T, D_T, P], bf16, tag="w1c", name=f"w1c{k}")
        w2cs[k] = w2_pool.tile([P, FC_T, D], bf16, tag="w2c", name=f"w2c{k}")
        q.dma_start(w1cs[k][:], w1p[k])
        q.dma_start(w2cs[k][:], w2p[k])

    # Startup DMAs, ordered by need-by time. The first GEMM1 chain needs
    # w1 chunk 0's f-tile 0 + Xt quarter 0 (~10us, right as the warm-ups
    # end) -- both lead the sync queue so nothing starves them. The first
    # GELU drain needs just the 16KB b1 slice, which rides the otherwise
    # idle scalar queue; the b2 broadcast half of cbt follows there.
    w1c0 = w1_pool.tile([P, FC_T, D_T, P], bf16, tag="w1c")
    w2c0 = w2_pool.tile([P, FC_T, D], bf16, tag="w2c")
    w1cs[0], w2cs[0] = w1c0, w2c0
    H_T = D_T // 2
    nc.scalar.dma_start(cbt[:, : F // P], cb[:, : F // P])
    nc.sync.dma_start(w1c0[:, ds(0, 1)], w1p[0, :, ds(0, 1)])
    nc.sync.dma_start(xt[:, 0, ds(0, H_T)], xtp[0, :, ds(0, H_T)])
    nc.sync.dma_start(xt[:, 0, ds(H_T, H_T)], xtp[0, :, ds(H_T, H_T)])
    nc.scalar.dma_start(cbt[:, F // P :], cb[:, F // P :])
    nc.sync.dma_start(w1c0[:, ds(1, FC_T - 1)], w1p[0, :, ds(1, FC_T - 1)])
    nc.sync.dma_start(xt[:, 1], xtp[1])
    nc.sync.dma_start(xt[:, 2], xtp[2])
    nc.sync.dma_start(xt[:, 3], xtp[3])
    nc.sync.dma_start(w2c0[:], w2p[0])

    act_fn = AFT.Tanh if ACT_FN == "tanh" else AFT.Gelu_apprx_tanh

    hts = {}
    for k in range(N_FC):
        # prefetch next chunk's weights early in this chunk's compute
        if k + 1 < N_FC:
            load_chunk(k + 1, nc.sync)
        w1c = w1cs[k]
        w1cs[k] = None

        # ---- GEMM1: ht[f, c] = gelu(sum_d W1[d, f]^T Xt[d, c] + b1[f]) ----
        ht = ht_pool.tile([P, FC_T, C], bf16, tag="ht", name=f"ht{k}")
        hts[k] = ht
        for qi in range(NQ):
            for fti in range(FC_T):
                ps = mm1_psum.tile([P, QW], f32, tag="mm1")
                for di in range(D_T):
                    nc.tensor.matmul(
                        ps[:],
                        lhsT=w1c[:, fti, di],
                        rhs=xt[:, qi, di],
                        start=(di == 0),
                        stop=(di == D_T - 1),
                    )
                ft_g = k * FC_T + fti
                nc.scalar.activation(
                    ht[:, fti, ds(qi * QW, QW)],
                    ps[:],
                    act_fn,
                    bias=b1t[:, ft_g : ft_g + 1],
                    scale=1.0,
                )

        # ---- GEMM2 on chunk pairs: Yacc[c, d] += sum_f ht^T W2 over the
        # pair's 8 f-tiles in one PSUM chain (halves the DVE drain count,
        # which otherwise rate-matches the PE and beats against it) ----
        if k % 2 == 0:
            continue
        pair = (k - 1, k)
        for ci in range(C_B):
            for dci in range(2):
                ps = mm2_psum.tile([P, QW], f32, tag="mm2")
                for kk in pair:
                    for fti in range(FC_T):
                        nc.tensor.matmul(
                            ps[:],
                            lhsT=hts[kk][:, fti, ds(ci * P, P)],
                            rhs=w2cs[kk][:, fti, ds(dci * QW, QW)],
                            start=(kk == pair[0] and fti == 0),
                            stop=(kk == pair[1] and fti == FC_T - 1),
                        )
                ya = yacc[:, ci, ds(dci * QW, QW)]
                if k == 1:
                    nc.vector.tensor_add(
                        out=ya, in0=ps[:], in1=b2b[:, ds(dci * QW, QW)]
                    )
                else:
                    nc.vector.tensor_add(out=ya, in0=ya, in1=ps[:])
                if k == N_FC - 1:
                    # row complete: write each d-half as soon as its drain
                    # lands, alternating queues -- halves the post-stream
                    # writeback on the critical path at the kernel tail
                    q = nc.scalar if dci == 0 else nc.sync
                    q.dma_start(
                        y[ds(ci * P, P), ds(dci * QW, QW)],
                        yacc[:, ci, ds(dci * QW, QW)],
                    )
        hts[k - 1] = hts[k] = None
        w2cs[k - 1] = w2cs[k] = None


_NC_CACHE = None


def build_bass():
    global _NC_CACHE
    if _NC_CACHE is not None:
        return _NC_CACHE
    nc = bacc.Bacc("TRN2", target_bir_lowering=False, debug=False)
    f32 = mybir.dt.float32
    bf16 = mybir.dt.bfloat16
    xtp = nc.dram_tensor("xtp", [NQ, P, D_T, QW], bf16, kind="ExternalInput").ap()
    w1p = nc.dram_tensor(
        "w1p", [N_FC, P, FC_T, D_T, P], bf16, kind="ExternalInput"
    ).ap()
    w2p = nc.dram_tensor("w2p", [N_FC, P, FC_T, D], bf16, kind="ExternalInput").ap()
    cb = nc.dram_tensor("cb", [P, F // P + D], f32, kind="ExternalInput").ap()
    y = nc.dram_tensor("y", [C, D], f32, kind="ExternalOutput").ap()
    with tile.TileContext(nc) as tc:
        with ExitStack() as ctx:
            _emit(ctx, tc, xtp, w1p, w2p, cb, y)
    nc.compile()
    _NC_CACHE = nc
    return nc


def _in_maps(inputs, w1, b1, w2, b2):
    bf = ml_dtypes.bfloat16
    maps = []
    for e in range(E):
        xs = inputs[e * C : (e + 1) * C].astype(bf)
        # xtp[q, p, di, c0] = x[q*QW+c0, di*P+p]
        xtp = np.ascontiguousarray(xs.T.reshape(D_T, P, NQ, QW).transpose(2, 1, 0, 3))
        # w1p[k, p, fti, di, f0] = w1[di*P+p, k*FC+fti*P+f0]
        w1p = np.ascontiguousarray(
            w1[e].astype(bf).reshape(D_T, P, N_FC, FC_T, P).transpose(2, 1, 3, 0, 4)
        )
        # w2p[k, p, fti, d] = w2[k*FC+fti*P+p, d]
        w2p = np.ascontiguousarray(
            w2[e].astype(bf).reshape(N_FC, FC_T, P, D).transpose(0, 2, 1, 3)
        )
        # cb = [b1 packed [P, F/P] | b2 broadcast [P, D]]
        cb = np.concatenate(
            [
                np.ascontiguousarray(b1[e].reshape(F // P, P).T),
                np.broadcast_to(b2[e], (P, D)),
            ],
            axis=1,
        ).astype(np.float32)
        maps.append({"xtp": xtp, "w1p": w1p, "w2p": w2p, "cb": cb})
    return maps


def kernel_run(inputs, w1, b1, w2, b2, trace=False, **trace_kwargs):
    """Run on 8 NeuronCores; returns (full_output [T, D], BassKernelResults)."""
    inputs = np.asarray(inputs, dtype=np.float32)
    w1 = np.asarray(w1, dtype=np.float32)
    b1 = np.asarray(b1, dtype=np.float32)
    w2 = np.asarray(w2, dtype=np.float32)
    b2 = np.asarray(b2, dtype=np.float32)
    nc = build_bass()
    res = run_bass_kernel_spmd(
        nc,
        _in_maps(inputs, w1, b1, w2, b2),
        core_ids=list(range(E)),
        trace=trace,
        **trace_kwargs,
    )
    out = np.concatenate([res.results[e]["y"] for e in range(E)], axis=0)
    return out, res


def kernel(inputs, w1, b1, w2, b2):
    out, _ = kernel_run(inputs, w1, b1, w2, b2, trace=False)
    return out


# revision 26
# speedup vs baseline: 1.0027x; 1.0027x over previous
"""Trainium2 Bass kernel for nn_LocalExperts (MoE expert-parallel FFN), v2.

Reference computation (per full input):
    x  [T=16384, D=1024] -> reshape [E=8, C=2048, D]
    h  = gelu(x @ w1[e] + b1[e])     w1 [E, D, F=4096]
    y  = h @ w2[e] + b2[e]           w2 [E, F, D]
    out[T, D]

Sharding: expert parallelism across 8 NeuronCores. Expert e's tokens are
exactly rows [e*C:(e+1)*C] of the input, so core e gets that token slice
plus w1[e], b1[e], w2[e], b2[e]. No collectives; outputs are concatenated
on the host.

Design notes (evolved from a 592us f32r two-pass baseline to ~460us):
  - Everything 16-bit on the wire: x/w1/w2 are cast to bf16 host-side
    (biases, PSUM accumulation and y stay f32; end-to-end rel err ~3e-3
    vs the 2e-2 gate). bf16 matmuls run the PE at full rate AND get the
    compiler's fast-weight-load, so LDWEIGHTS (~97ns) hides under the
    512-row stream -- f32r's ~187ns loads did not quite hide.
  - Host prep also pre-transposes x and packs x/w1/w2 chunk-major so
    every DMA lands 2-8KB contiguous per partition; small-packet DMAs
    (256B) measured only ~63 GB/s and starved the first GEMM chain.
  - Single token pass, F chunked 8 x 512: weights are DMA'd exactly once
    (20MB total HBM reads vs 76MB for the two-pass f32r version).
  - GEMM2 accumulates chunk PAIRS in one PSUM chain (8 f-tiles) before
    the DVE drains into Yacc: with per-chunk drains the DVE rate-matched
    the PE and beat against it (432ns PE stall every ~49 matmuls).
  - Startup: the ACT queue carries only the two tiny loads the first
    GELU drain needs (fused b1t|b2b consts + w1 chunk 0's first f-tile);
    bulk streams on sync. A few warm matmuls ramp the HAM clock
    (1.2->2.4GHz) while the first Xt quarter lands.
"""

import os
from contextlib import ExitStack

import ml_dtypes
import numpy as np

import concourse.bass as bass
import concourse.tile as tile
from concourse import bacc
from concourse import mybir
from concourse.bass import ds, ts
from concourse.bass_utils import run_bass_kernel_spmd

AFT = mybir.ActivationFunctionType

E = 8
D = 1024
F = 4096
T = 16384
C = T // E          # tokens per core
P = 128

FC = 512            # F chunk per iteration
N_FC = F // FC      # 8 chunks
FC_T = FC // P      # 4 f-tiles per chunk
D_T = D // P        # 8 d-tiles
C_B = C // P        # 16 token blocks
NQ = 4              # token quarters (GEMM1 moving dim 512)
QW = C // NQ
N_WARM = int(os.environ.get("KERNEL_WARM", "5"))  # HAM clock warm-up matmuls

# test-only: CoreSim lacks Gelu; "tanh" swaps the activation for sim gating
ACT_FN = os.environ.get("KERNEL_ACT", "gelu")


def _emit(ctx: ExitStack, tc: tile.TileContext, xtp, w1p, w2p, cb, y):
    nc = tc.nc
    f32 = mybir.dt.float32
    bf16 = mybir.dt.bfloat16

    consts = ctx.enter_context(tc.tile_pool(name="consts", bufs=1))
    xt_pool = ctx.enter_context(tc.tile_pool(name="xt", bufs=1))
    yacc_pool = ctx.enter_context(tc.tile_pool(name="yacc", bufs=1))
    w1_pool = ctx.enter_context(tc.tile_pool(name="w1c", bufs=2))
    w2_pool = ctx.enter_context(tc.tile_pool(name="w2c", bufs=3))
    ht_pool = ctx.enter_context(tc.tile_pool(name="ht", bufs=3))
    mm1_psum = ctx.enter_context(tc.tile_pool(name="mm1", bufs=3, space="PSUM"))
    mm2_psum = ctx.enter_context(tc.tile_pool(name="mm2", bufs=4, space="PSUM"))

    # Warm the PE HAM clock while the first Xt quarter + w1 chunk DMA in.
    if N_WARM:
        dummy = consts.tile([P, QW], bf16)
        nc.gpsimd.memset(dummy[:], 0.0)
        for _ in range(N_WARM):
            warm_ps = mm1_psum.tile([P, QW], f32, tag="mm1", name="warm_ps")
            nc.tensor.matmul(warm_ps[:], lhsT=dummy[:, :P], rhs=dummy[:],
                             start=True, stop=True)

    xt = xt_pool.tile([P, NQ, D_T, QW], bf16, tag="xt")
    yacc = yacc_pool.tile([P, C_B, D], f32, tag="yacc")
    # fused consts: [:, :F//P] = b1 packed per f-tile, [:, F//P:] = b2
    # broadcast -- one 4.2KB-per-partition DMA instead of two small ones.
    cbt = consts.tile([P, F // P + D], f32)
    b1t = cbt[:, : F // P]
    b2b = cbt[:, F // P :]

    w1cs = [None] * N_FC
    w2cs = [None] * N_FC

    def load_chunk(k, q):
        w1cs[k] = w1_pool.tile([P, FC_T, D_T, P], bf16, tag="w1c", name=f"w1c{k}")
        w2cs[k] = w2_pool.tile([P, FC_T, D], bf16, tag="w2c", name=f"w2c{k}")
        q.dma_start(w1cs[k][:], w1p[k])
        q.dma_start(w2cs[k][:], w2p[k])

    # Startup DMAs, ordered by need-by time on the sync queue so the
    # first GEMM1 chain streams continuously at DMA pace from ~10us --
    # any >1us hole in the PE stream triggers a HAM downclock that costs
    # far more than the hole. Xt quarter 0 lands in di-pair slices just
    # ahead of the chain's half-clock matmul pace; w1 chunk 0 arrives
    # f-tile by f-tile the same way. The first GELU drain needs just the
    # 16KB b1 slice, which rides the otherwise idle scalar queue.
    w1c0 = w1_pool.tile([P, FC_T, D_T, P], bf16, tag="w1c")
    w2c0 = w2_pool.tile([P, FC_T, D], bf16, tag="w2c")
    w1cs[0], w2cs[0] = w1c0, w2c0
    nc.scalar.dma_start(cbt[:, : F // P], cb[:, : F // P])
    nc.sync.dma_start(w1c0[:, ds(0, 1)], w1p[0, :, ds(0, 1)])
    nc.sync.dma_start(xt[:, 0, ds(0, 2)], xtp[0, :, ds(0, 2)])
    nc.sync.dma_start(xt[:, 0, ds(2, 2)], xtp[0, :, ds(2, 2)])
    nc.scalar.dma_start(cbt[:, F // P :], cb[:, F // P :])
    nc.sync.dma_start(w1c0[:, ds(1, 1)], w1p[0, :, ds(1, 1)])
    nc.sync.dma_start(xt[:, 0, ds(4, 2)], xtp[0, :, ds(4, 2)])
    nc.sync.dma_start(xt[:, 0, ds(6, 2)], xtp[0, :, ds(6, 2)])
    nc.sync.dma_start(w1c0[:, ds(2, 2)], w1p[0, :, ds(2, 2)])
    nc.sync.dma_start(xt[:, 1], xtp[1])
    nc.sync.dma_start(xt[:, 2], xtp[2])
    nc.sync.dma_start(xt[:, 3], xtp[3])
    nc.sync.dma_start(w2c0[:], w2p[0])

    act_fn = AFT.Tanh if ACT_FN == "tanh" else AFT.Gelu_apprx_tanh

    hts = {}
    for k in range(N_FC):
        # prefetch next chunk's weights early in this chunk's compute
        if k + 1 < N_FC:
            load_chunk(k + 1, nc.sync)
        w1c = w1cs[k]
        w1cs[k] = None

        # ---- GEMM1: ht[f, c] = gelu(sum_d W1[d, f]^T Xt[d, c] + b1[f]) ----
        ht = ht_pool.tile([P, FC_T, C], bf16, tag="ht", name=f"ht{k}")
        hts[k] = ht
        for qi in range(NQ):
            for fti in range(FC_T):
                ps = mm1_psum.tile([P, QW], f32, tag="mm1")
                for di in range(D_T):
                    nc.tensor.matmul(
                        ps[:],
                        lhsT=w1c[:, fti, di],
                        rhs=xt[:, qi, di],
                        start=(di == 0),
                        stop=(di == D_T - 1),
                    )
                ft_g = k * FC_T + fti
                nc.scalar.activation(
                    ht[:, fti, ds(qi * QW, QW)],
                    ps[:],
                    act_fn,
                    bias=b1t[:, ft_g : ft_g + 1],
                    scale=1.0,
                )

        # ---- GEMM2 on chunk pairs: Yacc[c, d] += sum_f ht^T W2 over the
        # pair's 8 f-tiles in one PSUM chain (halves the DVE drain count,
        # which otherwise rate-matches the PE and beats against it) ----
        if k % 2 == 0:
            continue
        pair = (k - 1, k)
        for ci in range(C_B):
            for dci in range(2):
                ps = mm2_psum.tile([P, QW], f32, tag="mm2")
                for kk in pair:
                    for fti in range(FC_T):
                        nc.tensor.matmul(
                            ps[:],
                            lhsT=hts[kk][:, fti, ds(ci * P, P)],
                            rhs=w2cs[kk][:, fti, ds(dci * QW, QW)],
                            start=(kk == pair[0] and fti == 0),
                            stop=(kk == pair[1] and fti == FC_T - 1),
                        )
                ya = yacc[:, ci, ds(dci * QW, QW)]
                if k == 1:
                    nc.vector.tensor_add(
                        out=ya, in0=ps[:], in1=b2b[:, ds(dci * QW, QW)]
                    )
                else:
                    nc.vector.tensor_add(out=ya, in0=ya, in1=ps[:])
                if k == N_FC - 1:
                    # row complete: write each d-half as soon as its drain
                    # lands, alternating queues -- halves the post-stream
                    # writeback on the critical path at the kernel tail
                    q = nc.scalar if dci == 0 else nc.sync
                    q.dma_start(
                        y[ds(ci * P, P), ds(dci * QW, QW)],
                        yacc[:, ci, ds(dci * QW, QW)],
                    )
        hts[k - 1] = hts[k] = None
        w2cs[k - 1] = w2cs[k] = None


_NC_CACHE = None


def build_bass():
    global _NC_CACHE
    if _NC_CACHE is not None:
        return _NC_CACHE
    nc = bacc.Bacc("TRN2", target_bir_lowering=False, debug=False)
    f32 = mybir.dt.float32
    bf16 = mybir.dt.bfloat16
    xtp = nc.dram_tensor("xtp", [NQ, P, D_T, QW], bf16, kind="ExternalInput").ap()
    w1p = nc.dram_tensor(
        "w1p", [N_FC, P, FC_T, D_T, P], bf16, kind="ExternalInput"
    ).ap()
    w2p = nc.dram_tensor("w2p", [N_FC, P, FC_T, D], bf16, kind="ExternalInput").ap()
    cb = nc.dram_tensor("cb", [P, F // P + D], f32, kind="ExternalInput").ap()
    y = nc.dram_tensor("y", [C, D], f32, kind="ExternalOutput").ap()
    with tile.TileContext(nc) as tc:
        with ExitStack() as ctx:
            _emit(ctx, tc, xtp, w1p, w2p, cb, y)
    nc.compile()
    _NC_CACHE = nc
    return nc


def _in_maps(inputs, w1, b1, w2, b2):
    bf = ml_dtypes.bfloat16
    maps = []
    for e in range(E):
        xs = inputs[e * C : (e + 1) * C].astype(bf)
        # xtp[q, p, di, c0] = x[q*QW+c0, di*P+p]
        xtp = np.ascontiguousarray(xs.T.reshape(D_T, P, NQ, QW).transpose(2, 1, 0, 3))
        # w1p[k, p, fti, di, f0] = w1[di*P+p, k*FC+fti*P+f0]
        w1p = np.ascontiguousarray(
            w1[e].astype(bf).reshape(D_T, P, N_FC, FC_T, P).transpose(2, 1, 3, 0, 4)
        )
        # w2p[k, p, fti, d] = w2[k*FC+fti*P+p, d]
        w2p = np.ascontiguousarray(
            w2[e].astype(bf).reshape(N_FC, FC_T, P, D).transpose(0, 2, 1, 3)
        )
        # cb = [b1 packed [P, F/P] | b2 broadcast [P, D]]
        cb = np.concatenate(
            [
                np.ascontiguousarray(b1[e].reshape(F // P, P).T),
                np.broadcast_to(b2[e], (P, D)),
            ],
            axis=1,
        ).astype(np.float32)
        maps.append({"xtp": xtp, "w1p": w1p, "w2p": w2p, "cb": cb})
    return maps


def kernel_run(inputs, w1, b1, w2, b2, trace=False, **trace_kwargs):
    """Run on 8 NeuronCores; returns (full_output [T, D], BassKernelResults)."""
    inputs = np.asarray(inputs, dtype=np.float32)
    w1 = np.asarray(w1, dtype=np.float32)
    b1 = np.asarray(b1, dtype=np.float32)
    w2 = np.asarray(w2, dtype=np.float32)
    b2 = np.asarray(b2, dtype=np.float32)
    nc = build_bass()
    res = run_bass_kernel_spmd(
        nc,
        _in_maps(inputs, w1, b1, w2, b2),
        core_ids=list(range(E)),
        trace=trace,
        **trace_kwargs,
    )
    out = np.concatenate([res.results[e]["y"] for e in range(E)], axis=0)
    return out, res


def kernel(inputs, w1, b1, w2, b2):
    out, _ = kernel_run(inputs, w1, b1, w2, b2, trace=False)
    return out


# revision 29
# speedup vs baseline: 1.0035x; 1.0008x over previous
"""Trainium2 Bass kernel for nn_LocalExperts (MoE expert-parallel FFN), v2.

Reference computation (per full input):
    x  [T=16384, D=1024] -> reshape [E=8, C=2048, D]
    h  = gelu(x @ w1[e] + b1[e])     w1 [E, D, F=4096]
    y  = h @ w2[e] + b2[e]           w2 [E, F, D]
    out[T, D]

Sharding: expert parallelism across 8 NeuronCores. Expert e's tokens are
exactly rows [e*C:(e+1)*C] of the input, so core e gets that token slice
plus w1[e], b1[e], w2[e], b2[e]. No collectives; outputs are concatenated
on the host.

Design notes (evolved from a 592us f32r two-pass baseline to ~460us):
  - Everything 16-bit on the wire: x/w1/w2 are cast to bf16 host-side
    (biases, PSUM accumulation and y stay f32; end-to-end rel err ~3e-3
    vs the 2e-2 gate). bf16 matmuls run the PE at full rate AND get the
    compiler's fast-weight-load, so LDWEIGHTS (~97ns) hides under the
    512-row stream -- f32r's ~187ns loads did not quite hide.
  - Host prep also pre-transposes x and packs x/w1/w2 chunk-major so
    every DMA lands 2-8KB contiguous per partition; small-packet DMAs
    (256B) measured only ~63 GB/s and starved the first GEMM chain.
  - Single token pass, F chunked 8 x 512: weights are DMA'd exactly once
    (20MB total HBM reads vs 76MB for the two-pass f32r version).
  - GEMM2 accumulates chunk PAIRS in one PSUM chain (8 f-tiles) before
    the DVE drains into Yacc: with per-chunk drains the DVE rate-matched
    the PE and beat against it (432ns PE stall every ~49 matmuls).
  - Startup: the ACT queue carries only the two tiny loads the first
    GELU drain needs (fused b1t|b2b consts + w1 chunk 0's first f-tile);
    bulk streams on sync. A few warm matmuls ramp the HAM clock
    (1.2->2.4GHz) while the first Xt quarter lands.
"""

import os
from contextlib import ExitStack

import ml_dtypes
import numpy as np

import concourse.bass as bass
import concourse.tile as tile
from concourse import bacc
from concourse import mybir
from concourse.bass import ds, ts
from concourse.bass_utils import run_bass_kernel_spmd

AFT = mybir.ActivationFunctionType

E = 8
D = 1024
F = 4096
T = 16384
C = T // E          # tokens per core
P = 128

FC = 512            # F chunk per iteration
N_FC = F // FC      # 8 chunks
FC_T = FC // P      # 4 f-tiles per chunk
D_T = D // P        # 8 d-tiles
C_B = C // P        # 16 token blocks
NQ = 4              # token quarters (GEMM1 moving dim 512)
QW = C // NQ
N_WARM = int(os.environ.get("KERNEL_WARM", "5"))  # HAM clock warm-up matmuls

# test-only: CoreSim lacks Gelu; "tanh" swaps the activation for sim gating
ACT_FN = os.environ.get("KERNEL_ACT", "gelu")


def _emit(ctx: ExitStack, tc: tile.TileContext, xtp, w1p, w2p, cb, y):
    nc = tc.nc
    f32 = mybir.dt.float32
    bf16 = mybir.dt.bfloat16

    consts = ctx.enter_context(tc.tile_pool(name="consts", bufs=1))
    xt_pool = ctx.enter_context(tc.tile_pool(name="xt", bufs=1))
    yacc_pool = ctx.enter_context(tc.tile_pool(name="yacc", bufs=1))
    w1_pool = ctx.enter_context(tc.tile_pool(name="w1c", bufs=2))
    w2_pool = ctx.enter_context(tc.tile_pool(name="w2c", bufs=3))
    ht_pool = ctx.enter_context(tc.tile_pool(name="ht", bufs=3))
    # One shared PSUM pool: all 8 banks rotate through GEMM1 quad-chains,
    # GEMM2 pair-chains and warm-ups. A chain group's banks are reused 8
    # allocations later, by which point their drains have long retired.
    mm_psum = ctx.enter_context(tc.tile_pool(name="mm", bufs=8, space="PSUM"))

    # Warm the PE HAM clock while the first Xt quarter + w1 chunk DMA in.
    if N_WARM:
        dummy = consts.tile([P, QW], bf16)
        nc.gpsimd.memset(dummy[:], 0.0)
        for _ in range(N_WARM):
            warm_ps = mm_psum.tile([P, QW], f32, tag="mm", name="warm_ps")
            nc.tensor.matmul(warm_ps[:], lhsT=dummy[:, :P], rhs=dummy[:],
                             start=True, stop=True)

    xt = xt_pool.tile([P, NQ, D_T, QW], bf16, tag="xt")
    yacc = yacc_pool.tile([P, C_B, D], f32, tag="yacc")
    # fused consts: [:, :F//P] = b1 packed per f-tile, [:, F//P:] = b2
    # broadcast -- one 4.2KB-per-partition DMA instead of two small ones.
    cbt = consts.tile([P, F // P + D], f32)
    b1t = cbt[:, : F // P]
    b2b = cbt[:, F // P :]

    w1cs = [None] * N_FC
    w2cs = [None] * N_FC

    def load_chunk(k, q):
        w1cs[k] = w1_pool.tile([P, FC_T, D_T, P], bf16, tag="w1c", name=f"w1c{k}")
        w2cs[k] = w2_pool.tile([P, FC_T, D], bf16, tag="w2c", name=f"w2c{k}")
        q.dma_start(w1cs[k][:], w1p[k])
        q.dma_start(w2cs[k][:], w2p[k])

    # Startup DMAs, ordered by need-by time on the sync queue so the
    # first GEMM1 chain streams continuously at DMA pace from ~10us --
    # any >1us hole in the PE stream triggers a HAM downclock that costs
    # far more than the hole. Xt quarter 0 lands in di-pair slices just
    # ahead of the chain's half-clock matmul pace; w1 chunk 0 arrives
    # f-tile by f-tile the same way. The first GELU drain needs just the
    # 16KB b1 slice, which rides the otherwise idle scalar queue.
    w1c0 = w1_pool.tile([P, FC_T, D_T, P], bf16, tag="w1c")
    w2c0 = w2_pool.tile([P, FC_T, D], bf16, tag="w2c")
    w1cs[0], w2cs[0] = w1c0, w2c0
    nc.scalar.dma_start(cbt[:, : F // P], cb[:, : F // P])
    nc.sync.dma_start(w1c0[:, ds(0, 1)], w1p[0, :, ds(0, 1)])
    nc.sync.dma_start(xt[:, 0, ds(0, 2)], xtp[0, :, ds(0, 2)])
    nc.sync.dma_start(xt[:, 0, ds(2, 2)], xtp[0, :, ds(2, 2)])
    nc.scalar.dma_start(cbt[:, F // P :], cb[:, F // P :])
    nc.sync.dma_start(w1c0[:, ds(1, 1)], w1p[0, :, ds(1, 1)])
    nc.sync.dma_start(xt[:, 0, ds(4, 2)], xtp[0, :, ds(4, 2)])
    nc.sync.dma_start(xt[:, 0, ds(6, 2)], xtp[0, :, ds(6, 2)])
    nc.sync.dma_start(w1c0[:, ds(2, 2)], w1p[0, :, ds(2, 2)])
    nc.sync.dma_start(xt[:, 1], xtp[1])
    nc.sync.dma_start(xt[:, 2], xtp[2])
    nc.sync.dma_start(xt[:, 3], xtp[3])
    nc.sync.dma_start(w2c0[:], w2p[0])

    act_fn = AFT.Tanh if ACT_FN == "tanh" else AFT.Gelu_apprx_tanh

    hts = {}
    for k in range(N_FC):
        # prefetch next chunk's weights early in this chunk's compute
        if k + 1 < N_FC:
            load_chunk(k + 1, nc.sync)
        w1c = w1cs[k]
        w1cs[k] = None

        # ---- GEMM1: ht[f, c] = gelu(sum_d W1[d, f]^T Xt[d, c] + b1[f]) ----
        # Chunk 0 runs quarter-sequential chains so the first chain can
        # stream behind the startup DMAs. Later chunks interleave all 4
        # quarter-chains per f-tile: 4 consecutive matmuls then share one
        # stationary tile, letting the codegen skip redundant LDWEIGHTS
        # (shrinks the tensor instruction stream -> fewer fetch stalls).
        ht = ht_pool.tile([P, FC_T, C], bf16, tag="ht", name=f"ht{k}")
        hts[k] = ht

        def g1_drain(fti, qi, ps):
            ft_g = k * FC_T + fti
            nc.scalar.activation(
                ht[:, fti, ds(qi * QW, QW)],
                ps[:],
                act_fn,
                bias=b1t[:, ft_g : ft_g + 1],
                scale=1.0,
            )

        if k == 0:
            for qi in range(NQ):
                for fti in range(FC_T):
                    ps = mm_psum.tile([P, QW], f32, tag="mm")
                    for di in range(D_T):
                        nc.tensor.matmul(
                            ps[:],
                            lhsT=w1c[:, fti, di],
                            rhs=xt[:, qi, di],
                            start=(di == 0),
                            stop=(di == D_T - 1),
                        )
                    g1_drain(fti, qi, ps)
        else:
            for fti in range(FC_T):
                pss = [
                    mm_psum.tile([P, QW], f32, tag="mm", name=f"g1ps{qi}")
                    for qi in range(NQ)
                ]
                for di in range(D_T):
                    for qi in range(NQ):
                        nc.tensor.matmul(
                            pss[qi][:],
                            lhsT=w1c[:, fti, di],
                            rhs=xt[:, qi, di],
                            start=(di == 0),
                            stop=(di == D_T - 1),
                        )
                for qi in range(NQ):
                    g1_drain(fti, qi, pss[qi])

        # ---- GEMM2 on chunk pairs: Yacc[c, d] += sum_f ht^T W2 over the
        # pair's 8 f-tiles in one PSUM chain (halves the DVE drain count,
        # which otherwise rate-matches the PE and beats against it) ----
        if k % 2 == 0:
            continue
        pair = (k - 1, k)
        for ci in range(C_B):
            # both d-half chains interleaved: consecutive matmul pairs
            # share the ht stationary tile (same LDWEIGHTS-dedup play)
            pss = [
                mm_psum.tile([P, QW], f32, tag="mm", name=f"g2ps{dci}")
                for dci in range(2)
            ]
            for kk in pair:
                for fti in range(FC_T):
                    for dci in range(2):
                        nc.tensor.matmul(
                            pss[dci][:],
                            lhsT=hts[kk][:, fti, ds(ci * P, P)],
                            rhs=w2cs[kk][:, fti, ds(dci * QW, QW)],
                            start=(kk == pair[0] and fti == 0),
                            stop=(kk == pair[1] and fti == FC_T - 1),
                        )
            for dci in range(2):
                ya = yacc[:, ci, ds(dci * QW, QW)]
                if k == 1:
                    nc.vector.tensor_add(
                        out=ya, in0=pss[dci][:], in1=b2b[:, ds(dci * QW, QW)]
                    )
                else:
                    nc.vector.tensor_add(out=ya, in0=ya, in1=pss[dci][:])
                if k == N_FC - 1:
                    # row complete: write each d-half as soon as its drain
                    # lands, alternating queues -- halves the post-stream
                    # writeback on the critical path at the kernel tail
                    q = nc.scalar if dci == 0 else nc.sync
                    q.dma_start(
                        y[ds(ci * P, P), ds(dci * QW, QW)],
                        yacc[:, ci, ds(dci * QW, QW)],
                    )
        hts[k - 1] = hts[k] = None
        w2cs[k - 1] = w2cs[k] = None


_NC_CACHE = None


def build_bass():
    global _NC_CACHE
    if _NC_CACHE is not None:
        return _NC_CACHE
    nc = bacc.Bacc("TRN2", target_bir_lowering=False, debug=False)
    f32 = mybir.dt.float32
    bf16 = mybir.dt.bfloat16
    xtp = nc.dram_tensor("xtp", [NQ, P, D_T, QW], bf16, kind="ExternalInput").ap()
    w1p = nc.dram_tensor(
        "w1p", [N_FC, P, FC_T, D_T, P], bf16, kind="ExternalInput"
    ).ap()
    w2p = nc.dram_tensor("w2p", [N_FC, P, FC_T, D], bf16, kind="ExternalInput").ap()
    cb = nc.dram_tensor("cb", [P, F // P + D], f32, kind="ExternalInput").ap()
    y = nc.dram_tensor("y", [C, D], f32, kind="ExternalOutput").ap()
    with tile.TileContext(nc) as tc:
        with ExitStack() as ctx:
            _emit(ctx, tc, xtp, w1p, w2p, cb, y)
    nc.compile()
    _NC_CACHE = nc
    return nc


def _in_maps(inputs, w1, b1, w2, b2):
    bf = ml_dtypes.bfloat16
    maps = []
    for e in range(E):
        xs = inputs[e * C : (e + 1) * C].astype(bf)
        # xtp[q, p, di, c0] = x[q*QW+c0, di*P+p]
        xtp = np.ascontiguousarray(xs.T.reshape(D_T, P, NQ, QW).transpose(2, 1, 0, 3))
        # w1p[k, p, fti, di, f0] = w1[di*P+p, k*FC+fti*P+f0]
        w1p = np.ascontiguousarray(
            w1[e].astype(bf).reshape(D_T, P, N_FC, FC_T, P).transpose(2, 1, 3, 0, 4)
        )
        # w2p[k, p, fti, d] = w2[k*FC+fti*P+p, d]
        w2p = np.ascontiguousarray(
            w2[e].astype(bf).reshape(N_FC, FC_T, P, D).transpose(0, 2, 1, 3)
        )
        # cb = [b1 packed [P, F/P] | b2 broadcast [P, D]]
        cb = np.concatenate(
            [
                np.ascontiguousarray(b1[e].reshape(F // P, P).T),
                np.broadcast_to(b2[e], (P, D)),
            ],
            axis=1,
        ).astype(np.float32)
        maps.append({"xtp": xtp, "w1p": w1p, "w2p": w2p, "cb": cb})
    return maps


def kernel_run(inputs, w1, b1, w2, b2, trace=False, **trace_kwargs):
    """Run on 8 NeuronCores; returns (full_output [T, D], BassKernelResults)."""
    inputs = np.asarray(inputs, dtype=np.float32)
    w1 = np.asarray(w1, dtype=np.float32)
    b1 = np.asarray(b1, dtype=np.float32)
    w2 = np.asarray(w2, dtype=np.float32)
    b2 = np.asarray(b2, dtype=np.float32)
    nc = build_bass()
    res = run_bass_kernel_spmd(
        nc,
        _in_maps(inputs, w1, b1, w2, b2),
        core_ids=list(range(E)),
        trace=trace,
        **trace_kwargs,
    )
    out = np.concatenate([res.results[e]["y"] for e in range(E)], axis=0)
    return out, res


def kernel(inputs, w1, b1, w2, b2):
    out, _ = kernel_run(inputs, w1, b1, w2, b2, trace=False)
    return out


# revision 30
# speedup vs baseline: 1.0069x; 1.0034x over previous
"""Trainium2 Bass kernel for nn_LocalExperts (MoE expert-parallel FFN), v2.

Reference computation (per full input):
    x  [T=16384, D=1024] -> reshape [E=8, C=2048, D]
    h  = gelu(x @ w1[e] + b1[e])     w1 [E, D, F=4096]
    y  = h @ w2[e] + b2[e]           w2 [E, F, D]
    out[T, D]

Sharding: expert parallelism across 8 NeuronCores. Expert e's tokens are
exactly rows [e*C:(e+1)*C] of the input, so core e gets that token slice
plus w1[e], b1[e], w2[e], b2[e]. No collectives; outputs are concatenated
on the host.

Design notes (evolved from a 592us f32r two-pass baseline to ~460us):
  - Everything 16-bit on the wire: x/w1/w2 are cast to bf16 host-side
    (biases, PSUM accumulation and y stay f32; end-to-end rel err ~3e-3
    vs the 2e-2 gate). bf16 matmuls run the PE at full rate AND get the
    compiler's fast-weight-load, so LDWEIGHTS (~97ns) hides under the
    512-row stream -- f32r's ~187ns loads did not quite hide.
  - Host prep also pre-transposes x and packs x/w1/w2 chunk-major so
    every DMA lands 2-8KB contiguous per partition; small-packet DMAs
    (256B) measured only ~63 GB/s and starved the first GEMM chain.
  - Single token pass, F chunked 8 x 512: weights are DMA'd exactly once
    (20MB total HBM reads vs 76MB for the two-pass f32r version).
  - GEMM2 accumulates chunk PAIRS in one PSUM chain (8 f-tiles) before
    the DVE drains into Yacc: with per-chunk drains the DVE rate-matched
    the PE and beat against it (432ns PE stall every ~49 matmuls).
  - Startup: the ACT queue carries only the two tiny loads the first
    GELU drain needs (fused b1t|b2b consts + w1 chunk 0's first f-tile);
    bulk streams on sync. A few warm matmuls ramp the HAM clock
    (1.2->2.4GHz) while the first Xt quarter lands.
"""

import os
from contextlib import ExitStack

import ml_dtypes
import numpy as np

import concourse.bass as bass
import concourse.tile as tile
from concourse import bacc
from concourse import mybir
from concourse.bass import ds, ts
from concourse.bass_utils import run_bass_kernel_spmd

AFT = mybir.ActivationFunctionType

E = 8
D = 1024
F = 4096
T = 16384
C = T // E          # tokens per core
P = 128

FC = 512            # F chunk per iteration
N_FC = F // FC      # 8 chunks
FC_T = FC // P      # 4 f-tiles per chunk
D_T = D // P        # 8 d-tiles
C_B = C // P        # 16 token blocks
NQ = 4              # token quarters (GEMM1 moving dim 512)
QW = C // NQ
N_WARM = int(os.environ.get("KERNEL_WARM", "5"))  # HAM clock warm-up matmuls

# test-only: CoreSim lacks Gelu; "tanh" swaps the activation for sim gating
ACT_FN = os.environ.get("KERNEL_ACT", "gelu")


def _emit(ctx: ExitStack, tc: tile.TileContext, xtp, w1p, w2p, cb, y):
    nc = tc.nc
    f32 = mybir.dt.float32
    bf16 = mybir.dt.bfloat16

    consts = ctx.enter_context(tc.tile_pool(name="consts", bufs=1))
    xt_pool = ctx.enter_context(tc.tile_pool(name="xt", bufs=1))
    yacc_pool = ctx.enter_context(tc.tile_pool(name="yacc", bufs=1))
    w1_pool = ctx.enter_context(tc.tile_pool(name="w1c", bufs=2))
    w2_pool = ctx.enter_context(tc.tile_pool(name="w2c", bufs=3))
    ht_pool = ctx.enter_context(tc.tile_pool(name="ht", bufs=3))
    # One shared PSUM pool: all 8 banks rotate through GEMM1 quad-chains,
    # GEMM2 pair-chains and warm-ups. A chain group's banks are reused 8
    # allocations later, by which point their drains have long retired.
    mm_psum = ctx.enter_context(tc.tile_pool(name="mm", bufs=8, space="PSUM"))

    # Warm the PE HAM clock while the first Xt quarter + w1 chunk DMA in.
    if N_WARM:
        dummy = consts.tile([P, QW], bf16)
        nc.gpsimd.memset(dummy[:], 0.0)
        for _ in range(N_WARM):
            warm_ps = mm_psum.tile([P, QW], f32, tag="mm", name="warm_ps")
            nc.tensor.matmul(warm_ps[:], lhsT=dummy[:, :P], rhs=dummy[:],
                             start=True, stop=True)

    xt = xt_pool.tile([P, NQ, D_T, QW], bf16, tag="xt")
    yacc = yacc_pool.tile([P, C_B, D], f32, tag="yacc")
    # fused consts: [:, :F//P] = b1 packed per f-tile, [:, F//P:] = b2
    # broadcast -- one 4.2KB-per-partition DMA instead of two small ones.
    cbt = consts.tile([P, F // P + D], f32)
    b1t = cbt[:, : F // P]
    b2b = cbt[:, F // P :]

    w1cs = [None] * N_FC
    w2cs = [None] * N_FC

    def load_chunk(k, q):
        w1cs[k] = w1_pool.tile([P, FC_T, D_T, P], bf16, tag="w1c", name=f"w1c{k}")
        w2cs[k] = w2_pool.tile([P, FC_T, D], bf16, tag="w2c", name=f"w2c{k}")
        q.dma_start(w1cs[k][:], w1p[k])
        q.dma_start(w2cs[k][:], w2p[k])

    # Startup DMAs, ordered by need-by time on the sync queue so the
    # first GEMM1 chain streams continuously at DMA pace from ~10us --
    # any >1us hole in the PE stream triggers a HAM downclock that costs
    # far more than the hole. Xt quarter 0 lands in di-pair slices just
    # ahead of the chain's half-clock matmul pace; w1 chunk 0 arrives
    # f-tile by f-tile the same way. The first GELU drain needs just the
    # 16KB b1 slice, which rides the otherwise idle scalar queue.
    w1c0 = w1_pool.tile([P, FC_T, D_T, P], bf16, tag="w1c")
    w2c0 = w2_pool.tile([P, FC_T, D], bf16, tag="w2c")
    w1cs[0], w2cs[0] = w1c0, w2c0
    nc.scalar.dma_start(cbt[:, : F // P], cb[:, : F // P])
    nc.sync.dma_start(w1c0[:, ds(0, 1)], w1p[0, :, ds(0, 1)])
    nc.sync.dma_start(xt[:, 0, ds(0, 2)], xtp[0, :, ds(0, 2)])
    nc.sync.dma_start(xt[:, 0, ds(2, 2)], xtp[0, :, ds(2, 2)])
    nc.sync.dma_start(xt[:, 0, ds(4, 2)], xtp[0, :, ds(4, 2)])
    nc.sync.dma_start(xt[:, 0, ds(6, 2)], xtp[0, :, ds(6, 2)])
    nc.sync.dma_start(w1c0[:, ds(1, 1)], w1p[0, :, ds(1, 1)])
    nc.sync.dma_start(xt[:, 1, ds(0, 4)], xtp[1, :, ds(0, 4)])
    nc.sync.dma_start(w1c0[:, ds(2, 2)], w1p[0, :, ds(2, 2)])
    nc.sync.dma_start(xt[:, 1, ds(4, 4)], xtp[1, :, ds(4, 4)])
    nc.sync.dma_start(xt[:, 2], xtp[2])
    nc.sync.dma_start(xt[:, 3], xtp[3])
    nc.scalar.dma_start(cbt[:, F // P :], cb[:, F // P :])
    nc.sync.dma_start(w2c0[:], w2p[0])

    act_fn = AFT.Tanh if ACT_FN == "tanh" else AFT.Gelu_apprx_tanh

    hts = {}
    for k in range(N_FC):
        # prefetch next chunk's weights early in this chunk's compute
        if k + 1 < N_FC:
            load_chunk(k + 1, nc.sync)
        w1c = w1cs[k]
        w1cs[k] = None

        # ---- GEMM1: ht[f, c] = gelu(sum_d W1[d, f]^T Xt[d, c] + b1[f]) ----
        # Chunk 0 runs quarter-sequential chains so the first chain can
        # stream behind the startup DMAs. Later chunks interleave all 4
        # quarter-chains per f-tile: 4 consecutive matmuls then share one
        # stationary tile, letting the codegen skip redundant LDWEIGHTS
        # (shrinks the tensor instruction stream -> fewer fetch stalls).
        ht = ht_pool.tile([P, FC_T, C], bf16, tag="ht", name=f"ht{k}")
        hts[k] = ht

        def g1_drain(fti, qi, ps):
            ft_g = k * FC_T + fti
            nc.scalar.activation(
                ht[:, fti, ds(qi * QW, QW)],
                ps[:],
                act_fn,
                bias=b1t[:, ft_g : ft_g + 1],
                scale=1.0,
            )

        if k == 0:
            for qi in range(NQ):
                for fti in range(FC_T):
                    ps = mm_psum.tile([P, QW], f32, tag="mm")
                    for di in range(D_T):
                        nc.tensor.matmul(
                            ps[:],
                            lhsT=w1c[:, fti, di],
                            rhs=xt[:, qi, di],
                            start=(di == 0),
                            stop=(di == D_T - 1),
                        )
                    g1_drain(fti, qi, ps)
        else:
            for fti in range(FC_T):
                pss = [
                    mm_psum.tile([P, QW], f32, tag="mm", name=f"g1ps{qi}")
                    for qi in range(NQ)
                ]
                for di in range(D_T):
                    for qi in range(NQ):
                        nc.tensor.matmul(
                            pss[qi][:],
                            lhsT=w1c[:, fti, di],
                            rhs=xt[:, qi, di],
                            start=(di == 0),
                            stop=(di == D_T - 1),
                        )
                for qi in range(NQ):
                    g1_drain(fti, qi, pss[qi])

        # ---- GEMM2 on chunk pairs: Yacc[c, d] += sum_f ht^T W2 over the
        # pair's 8 f-tiles in one PSUM chain (halves the DVE drain count,
        # which otherwise rate-matches the PE and beats against it) ----
        if k % 2 == 0:
            continue
        pair = (k - 1, k)
        for ci in range(C_B):
            # both d-half chains interleaved: consecutive matmul pairs
            # share the ht stationary tile (same LDWEIGHTS-dedup play)
            pss = [
                mm_psum.tile([P, QW], f32, tag="mm", name=f"g2ps{dci}")
                for dci in range(2)
            ]
            for kk in pair:
                for fti in range(FC_T):
                    for dci in range(2):
                        nc.tensor.matmul(
                            pss[dci][:],
                            lhsT=hts[kk][:, fti, ds(ci * P, P)],
                            rhs=w2cs[kk][:, fti, ds(dci * QW, QW)],
                            start=(kk == pair[0] and fti == 0),
                            stop=(kk == pair[1] and fti == FC_T - 1),
                        )
            for dci in range(2):
                ya = yacc[:, ci, ds(dci * QW, QW)]
                if k == 1:
                    nc.vector.tensor_add(
                        out=ya, in0=pss[dci][:], in1=b2b[:, ds(dci * QW, QW)]
                    )
                else:
                    nc.vector.tensor_add(out=ya, in0=ya, in1=pss[dci][:])
                if k == N_FC - 1:
                    # row complete: write each d-half as soon as its drain
                    # lands, alternating queues -- halves the post-stream
                    # writeback on the critical path at the kernel tail
                    q = nc.scalar if dci == 0 else nc.sync
                    q.dma_start(
                        y[ds(ci * P, P), ds(dci * QW, QW)],
                        yacc[:, ci, ds(dci * QW, QW)],
                    )
        hts[k - 1] = hts[k] = None
        w2cs[k - 1] = w2cs[k] = None


_NC_CACHE = None


def build_bass():
    global _NC_CACHE
    if _NC_CACHE is not None:
        return _NC_CACHE
    nc = bacc.Bacc("TRN2", target_bir_lowering=False, debug=False)
    f32 = mybir.dt.float32
    bf16 = mybir.dt.bfloat16
    xtp = nc.dram_tensor("xtp", [NQ, P, D_T, QW], bf16, kind="ExternalInput").ap()
    w1p = nc.dram_tensor(
        "w1p", [N_FC, P, FC_T, D_T, P], bf16, kind="ExternalInput"
    ).ap()
    w2p = nc.dram_tensor("w2p", [N_FC, P, FC_T, D], bf16, kind="ExternalInput").ap()
    cb = nc.dram_tensor("cb", [P, F // P + D], f32, kind="ExternalInput").ap()
    y = nc.dram_tensor("y", [C, D], f32, kind="ExternalOutput").ap()
    with tile.TileContext(nc) as tc:
        with ExitStack() as ctx:
            _emit(ctx, tc, xtp, w1p, w2p, cb, y)
    nc.compile()
    _NC_CACHE = nc
    return nc


def _in_maps(inputs, w1, b1, w2, b2):
    bf = ml_dtypes.bfloat16
    maps = []
    for e in range(E):
        xs = inputs[e * C : (e + 1) * C].astype(bf)
        # xtp[q, p, di, c0] = x[q*QW+c0, di*P+p]
        xtp = np.ascontiguousarray(xs.T.reshape(D_T, P, NQ, QW).transpose(2, 1, 0, 3))
        # w1p[k, p, fti, di, f0] = w1[di*P+p, k*FC+fti*P+f0]
        w1p = np.ascontiguousarray(
            w1[e].astype(bf).reshape(D_T, P, N_FC, FC_T, P).transpose(2, 1, 3, 0, 4)
        )
        # w2p[k, p, fti, d] = w2[k*FC+fti*P+p, d]
        w2p = np.ascontiguousarray(
            w2[e].astype(bf).reshape(N_FC, FC_T, P, D).transpose(0, 2, 1, 3)
        )
        # cb = [b1 packed [P, F/P] | b2 broadcast [P, D]]
        cb = np.concatenate(
            [
                np.ascontiguousarray(b1[e].reshape(F // P, P).T),
                np.broadcast_to(b2[e], (P, D)),
            ],
            axis=1,
        ).astype(np.float32)
        maps.append({"xtp": xtp, "w1p": w1p, "w2p": w2p, "cb": cb})
    return maps


def kernel_run(inputs, w1, b1, w2, b2, trace=False, **trace_kwargs):
    """Run on 8 NeuronCores; returns (full_output [T, D], BassKernelResults)."""
    inputs = np.asarray(inputs, dtype=np.float32)
    w1 = np.asarray(w1, dtype=np.float32)
    b1 = np.asarray(b1, dtype=np.float32)
    w2 = np.asarray(w2, dtype=np.float32)
    b2 = np.asarray(b2, dtype=np.float32)
    nc = build_bass()
    res = run_bass_kernel_spmd(
        nc,
        _in_maps(inputs, w1, b1, w2, b2),
        core_ids=list(range(E)),
        trace=trace,
        **trace_kwargs,
    )
    out = np.concatenate([res.results[e]["y"] for e in range(E)], axis=0)
    return out, res


def kernel(inputs, w1, b1, w2, b2):
    out, _ = kernel_run(inputs, w1, b1, w2, b2, trace=False)
    return out
